# revision 6
# baseline (speedup 1.0000x reference)
"""AUCM loss kernel for Trainium2 (8 NeuronCores, Bass/Tile).

Reference math (N = 16384 preds, int32 targets):
    pos = preds[targets==1]; neg = preds[targets==0]
    d_ij = 1 - (pos_i - neg_j)
    loss = mean_ij [ d_ij^2 + MARGIN*relu(d_ij) ]

Decomposition used here: with u_i = 1 - pos_i and v_j = neg_j, d_ij = u_i + v_j.
    sum_ij d^2     = Nv*sum(u^2) + 2*sum(u)*sum(v) + Nu*sum(v^2)   (rank-1 stats)
    sum_ij relu(d) = genuinely pairwise -> computed on device:
        TensorE rank-2 matmul  D = u (x) 1 + 1 (x) v  (lhsT=[2,128] {u,1},
        rhs=[2,FD] {1,v}) into PSUM, then a fused relu+row-sum in ONE
        instruction per tile, alternating between ScalarE
        (activation(Relu, accum_out=...)) and VectorE
        (tensor_scalar(max,0, accum_out=...)) so both engines consume PSUM
        tiles in parallel. Padding lanes carry -1e30 so relu() zeroes any
        pair touching padding; the rank-1 stats use zero-padded copies.

Sharding: the longer of (pos, neg) becomes the row side, split evenly across
the 8 cores (each core gets nblk 128-row blocks); the col side is replicated.
Each core DMAs out a [128, 6] partial-sum tile; the host combines.
"""

import math
import os
import sys

import numpy as np

for _p in ("/opt/trn_rl_repo", "/root/.axon_site/_ro/trn_rl_repo"):
    if os.path.isdir(_p) and _p not in sys.path:
        sys.path.append(_p)

import concourse.bacc as bacc
import concourse.bass as bass
import concourse.tile as tile
from concourse import mybir
from concourse.bass_utils import run_bass_kernel_spmd

N_CORES = 8
MARGIN = 1.0
NEG_BIG = -1.0e30
CHUNK = 2048  # free-dim per relu+accum instruction (4 PSUM banks max)

# test-harness hooks (the grading path never touches these)
TRACE = False
LAST_EXEC_NS = None
LAST_RESULTS = None

_prog_cache: dict = {}


def _chunks(q):
    out = []
    c0 = 0
    while c0 < q:
        fd = min(CHUNK, q - c0)
        out.append((c0, fd))
        c0 += fd
    return out


def _build(nblk, q):
    """Bass program for one core: nblk 128-row blocks x q cols."""
    key = (nblk, q)
    if key in _prog_cache:
        return _prog_cache[key]

    f32 = mybir.dt.float32
    chunks = _chunks(q)

    # greedy ACT/DVE assignment by modeled per-instruction cost (ns)
    def act_cost(fd):
        return (172.0 + fd) / 1.2

    def dve_cost(fd):
        return (120.0 + fd) / 0.96

    load_a = load_d = 0.0
    assign = []
    na = nd = 0
    for _b in range(nblk):
        for _c0, fd in chunks:
            if load_a + act_cost(fd) <= load_d + dve_cost(fd):
                assign.append(("A", na))
                na += 1
                load_a += act_cost(fd)
            else:
                assign.append(("D", nd))
                nd += 1
                load_d += dve_cost(fd)

    nc = bacc.Bacc(None, target_bir_lowering=False)
    rows_big = nc.dram_tensor("rows_big", [nblk * 128], f32, kind="ExternalInput")
    rows_zero = nc.dram_tensor("rows_zero", [nblk * 128], f32, kind="ExternalInput")
    cols_big = nc.dram_tensor("cols_big", [q], f32, kind="ExternalInput")
    cols_zero = nc.dram_tensor("cols_zero", [q], f32, kind="ExternalInput")
    out_t = nc.dram_tensor("out", [128, 6], f32, kind="ExternalOutput")

    max_fd = max(fd for _, fd in chunks)
    banks_per_tile = (max_fd + 511) // 512
    psum_bufs = max(2, min(4, 8 // banks_per_tile))

    with tile.TileContext(nc) as tc:
        with (
            tc.tile_pool(name="sb", bufs=1) as sb,
            tc.tile_pool(name="ps", bufs=psum_bufs, space="PSUM") as ps,
        ):
            # stationary operand: row 0 = u values (-1e30 padded), row 1 = ones
            w2 = sb.tile([2, nblk * 128], f32)
            nc.vector.memset(w2[:, :], 1.0)
            nc.sync.dma_start(
                out=w2[0:1, :], in_=rows_big[:].rearrange("(a x) -> a x", a=1)
            )
            # moving operand: row 0 = ones, row 1 = v values (-1e30 padded)
            rhs2 = sb.tile([2, q], f32)
            nc.vector.memset(rhs2[:, :], 1.0)
            nc.sync.dma_start(
                out=rhs2[1:2, :], in_=cols_big[:].rearrange("(a x) -> a x", a=1)
            )

            # zero-padded copies for the rank-1 stats
            az = sb.tile([128, nblk], f32)
            nc.sync.dma_start(
                out=az[:, :], in_=rows_zero[:].rearrange("(p a) -> p a", p=128)
            )
            qb = q // 128
            bz = sb.tile([128, qb], f32)
            nc.sync.dma_start(
                out=bz[:, :], in_=cols_zero[:].rearrange("(p a) -> p a", p=128)
            )

            acc_a = sb.tile([128, max(1, na)], f32)
            acc_d = sb.tile([128, max(1, nd)], f32)
            if na == 0:
                nc.vector.memset(acc_a[:, :], 0.0)
            if nd == 0:
                nc.vector.memset(acc_d[:, :], 0.0)

            k = 0
            for b in range(nblk):
                lhsT = w2[:, b * 128 : (b + 1) * 128]
                for c0, fd in chunks:
                    pt = ps.tile([128, fd], f32, tag="pt")
                    for j in range(0, fd, 512):
                        w = min(512, fd - j)
                        nc.tensor.matmul(
                            pt[:, j : j + w],
                            lhsT,
                            rhs2[:, c0 + j : c0 + j + w],
                            start=True,
                            stop=True,
                        )
                    eng, idx = assign[k]
                    k += 1
                    if eng == "A":
                        nc.scalar.activation(
                            pt[:, :fd],
                            pt[:, :fd],
                            mybir.ActivationFunctionType.Relu,
                            accum_out=acc_a[:, idx : idx + 1],
                        )
                    else:
                        nc.vector.tensor_scalar(
                            pt[:, :fd],
                            pt[:, :fd],
                            0.0,
                            None,
                            op0=mybir.AluOpType.max,
                            op1=mybir.AluOpType.add,
                            accum_out=acc_d[:, idx : idx + 1],
                        )

            out_sb = sb.tile([128, 6], f32)
            ax = mybir.AxisListType.X
            nc.vector.reduce_sum(out_sb[:, 0:1], acc_a[:, :], axis=ax)
            nc.vector.reduce_sum(out_sb[:, 1:2], acc_d[:, :], axis=ax)
            nc.vector.reduce_sum(out_sb[:, 2:3], az[:, :], axis=ax)
            az2 = sb.tile([128, nblk], f32)
            nc.vector.tensor_mul(az2[:, :], az[:, :], az[:, :])
            nc.vector.reduce_sum(out_sb[:, 3:4], az2[:, :], axis=ax)
            nc.vector.reduce_sum(out_sb[:, 4:5], bz[:, :], axis=ax)
            bz2 = sb.tile([128, qb], f32)
            nc.vector.tensor_mul(bz2[:, :], bz[:, :], bz[:, :])
            nc.vector.reduce_sum(out_sb[:, 5:6], bz2[:, :], axis=ax)
            nc.sync.dma_start(out=out_t[:, :], in_=out_sb[:, :])

    nc.finalize()
    _prog_cache[key] = nc
    return nc


def kernel(preds: np.ndarray, targets: np.ndarray) -> np.ndarray:
    global LAST_EXEC_NS, LAST_RESULTS

    p = np.asarray(preds, dtype=np.float32).reshape(-1)
    t = np.asarray(targets).reshape(-1)

    u = (1.0 - p[t == 1]).astype(np.float32)  # positive side
    v = p[t == 0].astype(np.float32)  # negative side
    nu, nv = u.size, v.size

    # Pick the row side (sharded across cores) to minimize per-core pair count.
    def cost(nrows, ncols):
        nblk = max(1, math.ceil(nrows / (128 * N_CORES)))
        q = max(512, 512 * math.ceil(ncols / 512))
        return nblk * 128 * q, nblk, q

    cost_u, nblk_u, q_u = cost(nu, nv)
    cost_v, nblk_v, q_v = cost(nv, nu)
    if cost_u <= cost_v:
        rows, cols, nblk, q = u, v, nblk_u, q_u
        n_rows_real, n_cols_real = nu, nv
    else:
        rows, cols, nblk, q = v, u, nblk_v, q_v
        n_rows_real, n_cols_real = nv, nu

    rtot = nblk * 128 * N_CORES
    rows_big = np.full(rtot, NEG_BIG, dtype=np.float32)
    rows_big[: rows.size] = rows
    rows_zero = np.zeros(rtot, dtype=np.float32)
    rows_zero[: rows.size] = rows
    cols_big = np.full(q, NEG_BIG, dtype=np.float32)
    cols_big[: cols.size] = cols
    cols_zero = np.zeros(q, dtype=np.float32)
    cols_zero[: cols.size] = cols

    nc = _build(nblk, q)

    per = nblk * 128
    in_maps = [
        {
            "rows_big": rows_big[c * per : (c + 1) * per],
            "rows_zero": rows_zero[c * per : (c + 1) * per],
            "cols_big": cols_big,
            "cols_zero": cols_zero,
        }
        for c in range(N_CORES)
    ]

    br = run_bass_kernel_spmd(nc, in_maps, list(range(N_CORES)), trace=TRACE)
    results = br.results
    LAST_EXEC_NS = getattr(br, "exec_time_ns", None)
    LAST_RESULTS = br

    outs = [np.asarray(r["out"], dtype=np.float64) for r in results]
    relu_sum = sum(o[:, 0].sum() + o[:, 1].sum() for o in outs)
    s_r1 = sum(o[:, 2].sum() for o in outs)
    s_r2 = sum(o[:, 3].sum() for o in outs)
    s_c1 = outs[0][:, 4].sum()
    s_c2 = outs[0][:, 5].sum()

    sq_sum = n_cols_real * s_r2 + 2.0 * s_r1 * s_c1 + n_rows_real * s_c2
    num_pairs = float(n_rows_real) * float(n_cols_real)
    with np.errstate(divide="ignore", invalid="ignore"):
        loss = np.float32(
            np.float64(sq_sum + MARGIN * relu_sum)
            / np.float64(num_pairs if num_pairs else np.float64(0.0))
        )
    return np.asarray(loss, dtype=np.float32)


# revision 7
# speedup vs baseline: 2.7512x; 2.7512x over previous
"""AUCM loss kernel for Trainium2 (8 NeuronCores, Bass/Tile).

Reference math (N = 16384 preds, int32 targets):
    pos = preds[targets==1]; neg = preds[targets==0]
    d_ij = 1 - (pos_i - neg_j)
    loss = mean_ij [ d_ij^2 + MARGIN*relu(d_ij) ]

Decomposition used here: with u_i = 1 - pos_i and v_j = neg_j, d_ij = u_i + v_j.
    sum_ij d^2     = Nv*sum(u^2) + 2*sum(u)*sum(v) + Nu*sum(v^2)   (rank-1 stats)
    sum_ij relu(d) = genuinely pairwise -> computed on device:
        TensorE rank-6 bf16 matmul D = (uh+um+ul) (x) 1 + 1 (x) (vh+vm+vl)
        into fp32 PSUM. Each side is split into 3 bf16 limbs on the host so
        D carries full fp32 accuracy; bf16 operands avoid walrus's 2-pass
        fp32 matmul lowering, and K=6 costs the same PE time as K=2 (PE
        cost is N cycles, K-independent). Each PSUM tile is then consumed
        by a fused relu+row-sum in ONE instruction, alternating between
        ScalarE (activation(Relu, accum_out=...)) and VectorE
        (tensor_scalar(max,0,add, accum_out=...)) so both engines drain
        PSUM in parallel. Padding lanes carry -1e30 in the hi limb so
        relu() kills any pair touching padding; the rank-1 stats use
        zero-padded fp32 copies.

Sharding: the longer of (pos, neg) becomes the row side, split evenly across
the 8 cores (each core gets nblk 128-row blocks); the col side is replicated.
Each core DMAs out a [128, 6] partial-sum tile; the host combines.
"""

import math
import os
import sys

import numpy as np

for _p in ("/opt/trn_rl_repo", "/root/.axon_site/_ro/trn_rl_repo"):
    if os.path.isdir(_p) and _p not in sys.path:
        sys.path.append(_p)

import ml_dtypes

import concourse.bacc as bacc
import concourse.tile as tile
from concourse import mybir
from concourse.bass_utils import run_bass_kernel_spmd

N_CORES = 8
MARGIN = 1.0
NEG_BIG = -1.0e30
CHUNK = 2048  # max free-dim per relu+accum instruction (4 PSUM banks)
MM_N = 512  # max free-dim per bf16 matmul into one-ish PSUM bank span
BF16 = ml_dtypes.bfloat16

# test-harness hooks (the grading path never touches these)
TRACE = False
LAST_EXEC_NS = None
LAST_RESULTS = None

_prog_cache: dict = {}


def _chunks(q):
    out = []
    c0 = 0
    while c0 < q:
        fd = min(CHUNK, q - c0)
        out.append((c0, fd))
        c0 += fd
    return out


def _split3(x32):
    """Split fp32 array into 3 bf16 limbs: x ~= h + m + l (fp32-accurate)."""
    h = x32.astype(BF16)
    r = x32 - h.astype(np.float32)
    m = r.astype(BF16)
    r2 = r - m.astype(np.float32)
    lo = r2.astype(BF16)
    return h, m, lo


def _build(nblk, q):
    """Bass program for one core: nblk 128-row blocks x q cols."""
    key = (nblk, q)
    if key in _prog_cache:
        return _prog_cache[key]

    f32 = mybir.dt.float32
    bf16 = mybir.dt.bfloat16
    chunks = _chunks(q)

    # greedy ACT/DVE assignment by HW-measured per-instruction cost (ns)
    def act_cost(fd):
        return (942.0 + fd) / 1.2

    def dve_cost(fd):
        return (497.0 + fd) / 0.96

    load_a = load_d = 0.0
    assign = []
    na = nd = 0
    for _b in range(nblk):
        for _c0, fd in chunks:
            if load_a + act_cost(fd) <= load_d + dve_cost(fd):
                assign.append(("A", na))
                na += 1
                load_a += act_cost(fd)
            else:
                assign.append(("D", nd))
                nd += 1
                load_d += dve_cost(fd)

    nc = bacc.Bacc(None, target_bir_lowering=False)
    rows3 = nc.dram_tensor("rows3", [3, nblk * 128], bf16, kind="ExternalInput")
    rows_zero = nc.dram_tensor("rows_zero", [nblk * 128], f32, kind="ExternalInput")
    cols3 = nc.dram_tensor("cols3", [3, q], bf16, kind="ExternalInput")
    cols_zero = nc.dram_tensor("cols_zero", [q], f32, kind="ExternalInput")
    out_t = nc.dram_tensor("out", [128, 6], f32, kind="ExternalOutput")

    max_fd = max(fd for _, fd in chunks)
    banks_per_tile = (max_fd * 4 + 2047) // 2048
    psum_bufs = max(2, min(4, 8 // banks_per_tile))

    with tile.TileContext(nc) as tc:
        with (
            tc.tile_pool(name="sb", bufs=1) as sb,
            tc.tile_pool(name="ps", bufs=psum_bufs, space="PSUM") as ps,
        ):
            # stationary operand: rows 0-2 = u limbs, rows 3-5 = ones
            w6 = sb.tile([6, nblk * 128], bf16)
            nc.vector.memset(w6[:, :], 1.0)
            nc.sync.dma_start(out=w6[0:3, :], in_=rows3[:, :])
            # moving operand: rows 0-2 = ones, rows 3-5 = v limbs
            rhs6 = sb.tile([6, q], bf16)
            nc.vector.memset(rhs6[:, :], 1.0)
            nc.sync.dma_start(out=rhs6[3:6, :], in_=cols3[:, :])

            # zero-padded fp32 copies for the rank-1 stats
            az = sb.tile([128, nblk], f32)
            nc.sync.dma_start(
                out=az[:, :], in_=rows_zero[:].rearrange("(p a) -> p a", p=128)
            )
            qb = q // 128
            bz = sb.tile([128, qb], f32)
            nc.sync.dma_start(
                out=bz[:, :], in_=cols_zero[:].rearrange("(p a) -> p a", p=128)
            )

            acc_a = sb.tile([128, max(1, na)], f32)
            acc_d = sb.tile([128, max(1, nd)], f32)
            if na == 0:
                nc.vector.memset(acc_a[:, :], 0.0)
            if nd == 0:
                nc.vector.memset(acc_d[:, :], 0.0)

            k = 0
            for b in range(nblk):
                lhsT = w6[:, b * 128 : (b + 1) * 128]
                for c0, fd in chunks:
                    pt = ps.tile([128, fd], f32, tag="pt")
                    for j in range(0, fd, MM_N):
                        w = min(MM_N, fd - j)
                        nc.tensor.matmul(
                            pt[:, j : j + w],
                            lhsT,
                            rhs6[:, c0 + j : c0 + j + w],
                            start=True,
                            stop=True,
                        )
                    eng, idx = assign[k]
                    k += 1
                    if eng == "A":
                        nc.scalar.activation(
                            pt[:, :fd],
                            pt[:, :fd],
                            mybir.ActivationFunctionType.Relu,
                            accum_out=acc_a[:, idx : idx + 1],
                        )
                    else:
                        nc.vector.tensor_scalar(
                            pt[:, :fd],
                            pt[:, :fd],
                            0.0,
                            None,
                            op0=mybir.AluOpType.max,
                            op1=mybir.AluOpType.add,
                            accum_out=acc_d[:, idx : idx + 1],
                        )

            out_sb = sb.tile([128, 6], f32)
            ax = mybir.AxisListType.X
            nc.vector.reduce_sum(out_sb[:, 0:1], acc_a[:, :], axis=ax)
            nc.vector.reduce_sum(out_sb[:, 1:2], acc_d[:, :], axis=ax)
            nc.vector.reduce_sum(out_sb[:, 2:3], az[:, :], axis=ax)
            az2 = sb.tile([128, nblk], f32)
            nc.vector.tensor_mul(az2[:, :], az[:, :], az[:, :])
            nc.vector.reduce_sum(out_sb[:, 3:4], az2[:, :], axis=ax)
            nc.vector.reduce_sum(out_sb[:, 4:5], bz[:, :], axis=ax)
            bz2 = sb.tile([128, qb], f32)
            nc.vector.tensor_mul(bz2[:, :], bz[:, :], bz[:, :])
            nc.vector.reduce_sum(out_sb[:, 5:6], bz2[:, :], axis=ax)
            nc.sync.dma_start(out=out_t[:, :], in_=out_sb[:, :])

    nc.finalize()
    _prog_cache[key] = nc
    return nc


def kernel(preds: np.ndarray, targets: np.ndarray) -> np.ndarray:
    global LAST_EXEC_NS, LAST_RESULTS

    p = np.asarray(preds, dtype=np.float32).reshape(-1)
    t = np.asarray(targets).reshape(-1)

    u = (1.0 - p[t == 1]).astype(np.float32)  # positive side
    v = p[t == 0].astype(np.float32)  # negative side
    nu, nv = u.size, v.size

    # Pick the row side (sharded across cores) to minimize per-core pair count.
    def cost(nrows, ncols):
        nblk = max(1, math.ceil(nrows / (128 * N_CORES)))
        q = max(128, 128 * math.ceil(ncols / 128))
        return nblk * 128 * q, nblk, q

    cost_u, nblk_u, q_u = cost(nu, nv)
    cost_v, nblk_v, q_v = cost(nv, nu)
    if cost_u <= cost_v:
        rows, cols, nblk, q = u, v, nblk_u, q_u
        n_rows_real, n_cols_real = nu, nv
    else:
        rows, cols, nblk, q = v, u, nblk_v, q_v
        n_rows_real, n_cols_real = nv, nu

    rtot = nblk * 128 * N_CORES
    rows_pad = np.zeros(rtot, dtype=np.float32)
    rows_pad[: rows.size] = rows
    cols_pad = np.zeros(q, dtype=np.float32)
    cols_pad[: cols.size] = cols

    rows3 = np.stack(_split3(rows_pad))  # [3, rtot] bf16
    rows3[0, rows.size :] = BF16(NEG_BIG)
    cols3 = np.stack(_split3(cols_pad))  # [3, q] bf16
    cols3[0, cols.size :] = BF16(NEG_BIG)

    nc = _build(nblk, q)

    per = nblk * 128
    in_maps = [
        {
            "rows3": np.ascontiguousarray(rows3[:, c * per : (c + 1) * per]),
            "rows_zero": rows_pad[c * per : (c + 1) * per],
            "cols3": cols3,
            "cols_zero": cols_pad,
        }
        for c in range(N_CORES)
    ]

    br = run_bass_kernel_spmd(nc, in_maps, list(range(N_CORES)), trace=TRACE)
    results = br.results
    LAST_EXEC_NS = getattr(br, "exec_time_ns", None)
    LAST_RESULTS = br

    outs = [np.asarray(r["out"], dtype=np.float64) for r in results]
    relu_sum = sum(o[:, 0].sum() + o[:, 1].sum() for o in outs)
    s_r1 = sum(o[:, 2].sum() for o in outs)
    s_r2 = sum(o[:, 3].sum() for o in outs)
    s_c1 = outs[0][:, 4].sum()
    s_c2 = outs[0][:, 5].sum()

    sq_sum = n_cols_real * s_r2 + 2.0 * s_r1 * s_c1 + n_rows_real * s_c2
    num_pairs = np.float64(n_rows_real) * np.float64(n_cols_real)
    with np.errstate(divide="ignore", invalid="ignore"):
        loss = np.float32(np.float64(sq_sum + MARGIN * relu_sum) / num_pairs)
    return np.asarray(loss, dtype=np.float32)


# revision 8
# speedup vs baseline: 2.9193x; 1.0611x over previous
"""AUCM loss kernel for Trainium2 (8 NeuronCores, Bass/Tile).

Reference math (N = 16384 preds, int32 targets):
    pos = preds[targets==1]; neg = preds[targets==0]
    d_ij = 1 - (pos_i - neg_j)
    loss = mean_ij [ d_ij^2 + MARGIN*relu(d_ij) ]

Decomposition used here: with u_i = 1 - pos_i and v_j = neg_j, d_ij = u_i + v_j.
    sum_ij d^2     = Nv*sum(u^2) + 2*sum(u)*sum(v) + Nu*sum(v^2)   (rank-1 stats)
    sum_ij relu(d) = genuinely pairwise -> computed on device:
        TensorE rank-6 bf16 matmul D = (uh+um+ul) (x) 1 + 1 (x) (vh+vm+vl)
        into fp32 PSUM. Each value is split into 3 bf16 limbs on the host so
        D carries fp32 accuracy; bf16 operands avoid walrus's 2-pass fp32
        matmul lowering, and K=6 costs the same PE time as K=2 (PE cost is
        N cycles, K-independent). Each PSUM tile is then consumed by a
        fused relu+row-sum in ONE instruction, alternating between ScalarE
        (activation(Relu, accum_out=...)) and VectorE
        (tensor_scalar(max,0,add, accum_out=...)) so both engines drain
        PSUM in parallel. Padding lanes carry -1e30 in the hi limb so
        relu() kills any pair touching padding; the rank-1 stats use
        zero-padded fp32 copies. A dummy-matmul burst at kernel start
        (overlapping the input DMAs) warms the PE HAM clock gate to
        2.4 GHz before the real matmuls issue.

Sharding: the longer of (pos, neg) becomes the row side, split evenly across
the 8 cores (each core gets nblk 128-row blocks); the col side is replicated.
Each core DMAs out a [128, 6] partial-sum tile; the host combines.
"""

import math
import os
import sys

import numpy as np

for _p in ("/opt/trn_rl_repo", "/root/.axon_site/_ro/trn_rl_repo"):
    if os.path.isdir(_p) and _p not in sys.path:
        sys.path.append(_p)

import ml_dtypes

import concourse.bacc as bacc
import concourse.tile as tile
from concourse import mybir
from concourse.bass_utils import run_bass_kernel_spmd

N_CORES = 8
MARGIN = 1.0
NEG_BIG = -1.0e30
CHUNK = 2048  # max free-dim per relu+accum instruction (4 PSUM banks)
MM_N = 512  # max free-dim per matmul (one PSUM bank of fp32)
N_WARMUP = 10  # dummy matmuls to warm the PE HAM gate (~4.3us cold)
BF16 = ml_dtypes.bfloat16

# test-harness hooks (the grading path never touches these)
TRACE = False
LAST_EXEC_NS = None
LAST_RESULTS = None

_prog_cache: dict = {}


def _chunks(q):
    out = []
    c0 = 0
    while c0 < q:
        fd = min(CHUNK, q - c0)
        out.append((c0, fd))
        c0 += fd
    return out


def _split3(x32):
    """Split fp32 array into 3 bf16 limbs: x ~= h + m + l (fp32-accurate)."""
    h = x32.astype(BF16)
    r = x32 - h.astype(np.float32)
    m = r.astype(BF16)
    r2 = r - m.astype(np.float32)
    lo = r2.astype(BF16)
    return h, m, lo


def _build(nblk, q):
    """Bass program for one core: nblk 128-row blocks x q cols."""
    key = (nblk, q)
    if key in _prog_cache:
        return _prog_cache[key]

    f32 = mybir.dt.float32
    bf16 = mybir.dt.bfloat16
    chunks = _chunks(q)

    # greedy ACT/DVE assignment by HW-measured per-instruction cost (ns)
    def act_cost(fd):
        return (281.0 + fd) / 1.2 + 250.0  # ACTIVATE + READ_ACCUMULATOR

    def dve_cost(fd):
        return (159.0 + fd) / 0.96

    load_a = load_d = 0.0
    assign = []
    na = nd = 0
    for _b in range(nblk):
        for _c0, fd in chunks:
            if load_a + act_cost(fd) <= load_d + dve_cost(fd):
                assign.append(("A", na))
                na += 1
                load_a += act_cost(fd)
            else:
                assign.append(("D", nd))
                nd += 1
                load_d += dve_cost(fd)

    nc = bacc.Bacc(None, target_bir_lowering=False)
    rows6 = nc.dram_tensor("rows6", [6, nblk * 128], bf16, kind="ExternalInput")
    rows_zero = nc.dram_tensor("rows_zero", [nblk * 128], f32, kind="ExternalInput")
    cols6 = nc.dram_tensor("cols6", [6, q], bf16, kind="ExternalInput")
    cols_zero = nc.dram_tensor("cols_zero", [q], f32, kind="ExternalInput")
    out_t = nc.dram_tensor("out", [128, 6], f32, kind="ExternalOutput")

    max_fd = max(fd for _, fd in chunks)
    banks_per_tile = (max_fd * 4 + 2047) // 2048
    psum_bufs = max(2, min(4, 7 // banks_per_tile))

    with tile.TileContext(nc) as tc:
        with (
            tc.tile_pool(name="sb", bufs=1) as sb,
            tc.tile_pool(name="ps", bufs=psum_bufs, space="PSUM") as ps,
            tc.tile_pool(name="warm", bufs=1, space="PSUM") as warm_pool,
        ):
            # PE warm-up: dummy matmuls with no input dependency, overlapping
            # the input DMAs. Content is irrelevant; sums land in a dead tile.
            wdummy = sb.tile([1, 128], bf16)
            rdummy = sb.tile([1, 512], bf16)
            nc.gpsimd.memset(wdummy[:, :], 1.0)
            nc.gpsimd.memset(rdummy[:, :], 1.0)
            warm_pt = warm_pool.tile([128, 512], f32)
            for _ in range(N_WARMUP):
                nc.tensor.matmul(
                    warm_pt[:, :], wdummy[:, :], rdummy[:, :], start=True, stop=True
                )

            # stationary operand: rows 0-2 = u limbs, rows 3-5 = ones
            w6 = sb.tile([6, nblk * 128], bf16)
            nc.sync.dma_start(out=w6[:, :], in_=rows6[:, :])
            # moving operand: rows 0-2 = ones, rows 3-5 = v limbs
            rhs6 = sb.tile([6, q], bf16)
            nc.sync.dma_start(out=rhs6[:, :], in_=cols6[:, :])

            # rank-1 stats on zero-padded fp32 copies (scheduled early; they
            # hide under the pipeline fill)
            out_sb = sb.tile([128, 6], f32)
            ax = mybir.AxisListType.X
            az = sb.tile([128, nblk], f32)
            nc.sync.dma_start(
                out=az[:, :], in_=rows_zero[:].rearrange("(p a) -> p a", p=128)
            )
            qb = q // 128
            bz = sb.tile([128, qb], f32)
            nc.sync.dma_start(
                out=bz[:, :], in_=cols_zero[:].rearrange("(p a) -> p a", p=128)
            )
            nc.vector.reduce_sum(out_sb[:, 2:3], az[:, :], axis=ax)
            az2 = sb.tile([128, nblk], f32)
            nc.vector.tensor_mul(az2[:, :], az[:, :], az[:, :])
            nc.vector.reduce_sum(out_sb[:, 3:4], az2[:, :], axis=ax)
            nc.vector.reduce_sum(out_sb[:, 4:5], bz[:, :], axis=ax)
            bz2 = sb.tile([128, qb], f32)
            nc.vector.tensor_mul(bz2[:, :], bz[:, :], bz[:, :])
            nc.vector.reduce_sum(out_sb[:, 5:6], bz2[:, :], axis=ax)

            acc_a = sb.tile([128, max(1, na)], f32)
            acc_d = sb.tile([128, max(1, nd)], f32)
            if na == 0:
                nc.gpsimd.memset(acc_a[:, :], 0.0)
            if nd == 0:
                nc.gpsimd.memset(acc_d[:, :], 0.0)

            k = 0
            for b in range(nblk):
                lhsT = w6[:, b * 128 : (b + 1) * 128]
                for c0, fd in chunks:
                    pt = ps.tile([128, fd], f32, tag="pt")
                    for j in range(0, fd, MM_N):
                        w = min(MM_N, fd - j)
                        nc.tensor.matmul(
                            pt[:, j : j + w],
                            lhsT,
                            rhs6[:, c0 + j : c0 + j + w],
                            start=True,
                            stop=True,
                        )
                    eng, idx = assign[k]
                    k += 1
                    if eng == "A":
                        nc.scalar.activation(
                            pt[:, :fd],
                            pt[:, :fd],
                            mybir.ActivationFunctionType.Relu,
                            accum_out=acc_a[:, idx : idx + 1],
                        )
                    else:
                        nc.vector.tensor_scalar(
                            pt[:, :fd],
                            pt[:, :fd],
                            0.0,
                            None,
                            op0=mybir.AluOpType.max,
                            op1=mybir.AluOpType.add,
                            accum_out=acc_d[:, idx : idx + 1],
                        )

            nc.vector.reduce_sum(out_sb[:, 0:1], acc_a[:, :], axis=ax)
            nc.vector.reduce_sum(out_sb[:, 1:2], acc_d[:, :], axis=ax)
            nc.sync.dma_start(out=out_t[:, :], in_=out_sb[:, :])

    nc.finalize()
    _prog_cache[key] = nc
    return nc


def kernel(preds: np.ndarray, targets: np.ndarray) -> np.ndarray:
    global LAST_EXEC_NS, LAST_RESULTS

    p = np.asarray(preds, dtype=np.float32).reshape(-1)
    t = np.asarray(targets).reshape(-1)

    u = (1.0 - p[t == 1]).astype(np.float32)  # positive side
    v = p[t == 0].astype(np.float32)  # negative side
    nu, nv = u.size, v.size

    # Pick the row side (sharded across cores) to minimize per-core pair count.
    def cost(nrows, ncols):
        nblk = max(1, math.ceil(nrows / (128 * N_CORES)))
        q = max(128, 128 * math.ceil(ncols / 128))
        return nblk * 128 * q, nblk, q

    cost_u, nblk_u, q_u = cost(nu, nv)
    cost_v, nblk_v, q_v = cost(nv, nu)
    if cost_u <= cost_v:
        rows, cols, nblk, q = u, v, nblk_u, q_u
        n_rows_real, n_cols_real = nu, nv
    else:
        rows, cols, nblk, q = v, u, nblk_v, q_v
        n_rows_real, n_cols_real = nv, nu

    rtot = nblk * 128 * N_CORES
    rows_pad = np.zeros(rtot, dtype=np.float32)
    rows_pad[: rows.size] = rows
    cols_pad = np.zeros(q, dtype=np.float32)
    cols_pad[: cols.size] = cols

    ones_r = np.ones(rtot, dtype=BF16)
    ones_c = np.ones(q, dtype=BF16)
    rh, rm, rl = _split3(rows_pad)
    rh[rows.size :] = BF16(NEG_BIG)
    ch, cm, cl = _split3(cols_pad)
    ch[cols.size :] = BF16(NEG_BIG)
    rows6 = np.stack([rh, rm, rl, ones_r, ones_r, ones_r])  # [6, rtot]
    # rows of rhs6: first 3 must be ones (pairing with u limbs), last 3 = v limbs
    cols6 = np.stack([ones_c, ones_c, ones_c, ch, cm, cl])  # [6, q]
    # rows6 rows 3-5 are ones but only rows 0-2 carry u; the ones rows pair
    # with cols6 limbs. (lhsT row k multiplies rhs row k.)

    nc = _build(nblk, q)

    per = nblk * 128
    in_maps = [
        {
            "rows6": np.ascontiguousarray(rows6[:, c * per : (c + 1) * per]),
            "rows_zero": rows_pad[c * per : (c + 1) * per],
            "cols6": cols6,
            "cols_zero": cols_pad,
        }
        for c in range(N_CORES)
    ]

    br = run_bass_kernel_spmd(nc, in_maps, list(range(N_CORES)), trace=TRACE)
    results = br.results
    LAST_EXEC_NS = getattr(br, "exec_time_ns", None)
    LAST_RESULTS = br

    outs = [np.asarray(r["out"], dtype=np.float64) for r in results]
    relu_sum = sum(o[:, 0].sum() + o[:, 1].sum() for o in outs)
    s_r1 = sum(o[:, 2].sum() for o in outs)
    s_r2 = sum(o[:, 3].sum() for o in outs)
    s_c1 = outs[0][:, 4].sum()
    s_c2 = outs[0][:, 5].sum()

    sq_sum = n_cols_real * s_r2 + 2.0 * s_r1 * s_c1 + n_rows_real * s_c2
    num_pairs = np.float64(n_rows_real) * np.float64(n_cols_real)
    with np.errstate(divide="ignore", invalid="ignore"):
        loss = np.float32(np.float64(sq_sum + MARGIN * relu_sum) / num_pairs)
    return np.asarray(loss, dtype=np.float32)


# revision 10
# speedup vs baseline: 3.0648x; 1.0498x over previous
"""AUCM loss kernel for Trainium2 (8 NeuronCores, Bass/Tile).

Reference math (N = 16384 preds, int32 targets):
    pos = preds[targets==1]; neg = preds[targets==0]
    d_ij = 1 - (pos_i - neg_j)
    loss = mean_ij [ d_ij^2 + MARGIN*relu(d_ij) ]

Decomposition: with u_i = 1 - pos_i and v_j = neg_j, d_ij = u_i + v_j.
    sum_ij d^2     = Nv*sum(u^2) + 2*sum(u)*sum(v) + Nu*sum(v^2)  (host, O(N))
    sum_ij relu(d) = the real O(Nu*Nv) work -> computed on device.

Device strategy (no TensorEngine at all — ScalarE and VectorE both stream the
pairwise grid directly out of SBUF, which beats producing D tiles on the PE
since the PE's cold-clock column rate equals just one consumer's rate):
  - v is DMA-broadcast to all 128 partitions: v_rep [128, q] fp32.
  - A 128-row block of u lives as one column u_col [128,1].
  - ScalarE, one instruction per block:
        ACTIVATE(Relu, in=v_rep, bias=u_col, accum_out)  ->
        per-partition sum_j relu(v_j + u_p); -1e30 padding (rows or cols)
        makes relu() return 0 for any padded pair.
  - VectorE, one instruction per block, using
        relu(v + u) = u + max(v, -u):
        TENSOR_SCALAR(max, scalar1=-u_col, reduce=add, accum_out) ->
        per-partition sum_j max(v_j, -u_p). Single-source fp32 SBUF runs in
        the DVE 2x port mode (2 elem/lane/cyc). The host adds the q*u_p
        correction in float64 (padded cols contribute max(-1e30,-u) = -u,
        cancelled exactly by +u; padded rows are dropped on the host).
  - Each block's accum lands in its own column of acc_a/acc_d; both matrices
    are DMA'd out raw [128, na+nd] and the host does the final combine.

Sharding: the longer of (pos, neg) becomes the row side, split evenly across
the 8 cores (each core gets nblk 128-row blocks); the col side is replicated.
"""

import math
import os
import sys

import numpy as np

for _p in ("/opt/trn_rl_repo", "/root/.axon_site/_ro/trn_rl_repo"):
    if os.path.isdir(_p) and _p not in sys.path:
        sys.path.append(_p)

import concourse.bacc as bacc
import concourse.bass as bass
import concourse.tile as tile
from concourse import mybir
from concourse.bass_utils import run_bass_kernel_spmd

N_CORES = 8
MARGIN = 1.0
NEG_BIG = -1.0e30
CHUNK = 4096  # max free-dim per consumer instruction

# test-harness hooks (the grading path never touches these)
TRACE = False
LAST_EXEC_NS = None
LAST_RESULTS = None

_prog_cache: dict = {}


def _chunks(q):
    out = []
    c0 = 0
    while c0 < q:
        fd = min(CHUNK, q - c0)
        out.append((c0, fd))
        c0 += fd
    return out


def _assign(nblk, chunks):
    """Greedy ACT/DVE split by modeled per-instruction cost (ns)."""

    def act_cost(fd):
        return (224.0 + fd) / 1.2 + 250.0  # ACTIVATE + READ_ACCUMULATOR

    def dve_cost(fd):
        return (58.0 + fd / 2.0) / 0.96  # 2x port mode

    load_a = load_d = 0.0
    assign = []
    na = nd = 0
    for _b in range(nblk):
        for _c0, fd in chunks:
            if load_a + act_cost(fd) <= load_d + dve_cost(fd):
                assign.append(("A", na))
                na += 1
                load_a += act_cost(fd)
            else:
                assign.append(("D", nd))
                nd += 1
                load_d += dve_cost(fd)
    return assign, na, nd


def _build(nblk, q):
    """Bass program for one core: nblk 128-row blocks x q cols."""
    key = (nblk, q)
    if key in _prog_cache:
        return _prog_cache[key]

    f32 = mybir.dt.float32
    bf16 = mybir.dt.bfloat16
    chunks = _chunks(q)
    assign, na, nd = _assign(nblk, chunks)
    na_c, nd_c = max(1, na), max(1, nd)

    nc = bacc.Bacc(None, target_bir_lowering=False)
    cols_big = nc.dram_tensor("cols_big", [q], f32, kind="ExternalInput")
    ub_t = nc.dram_tensor("ub", [128, nblk], f32, kind="ExternalInput")
    negu_t = nc.dram_tensor("negu", [128, nblk], f32, kind="ExternalInput")
    out_t = nc.dram_tensor("out", [128, na_c + nd_c], f32, kind="ExternalOutput")

    with tile.TileContext(nc) as tc:
        with tc.tile_pool(name="sb", bufs=1) as sb:
            v_rep = sb.tile([128, q], f32)
            h = cols_big[:]
            bc = bass.AP(tensor=h.tensor, offset=h.offset, ap=[[0, 128]] + list(h.ap))
            nc.sync.dma_start(out=v_rep[:, :], in_=bc)
            ub = sb.tile([128, nblk], f32)
            nc.sync.dma_start(out=ub[:, :], in_=ub_t[:, :])
            negu = sb.tile([128, nblk], f32)
            nc.sync.dma_start(out=negu[:, :], in_=negu_t[:, :])

            acc_a = sb.tile([128, na_c], f32)
            acc_d = sb.tile([128, nd_c], f32)
            if na == 0:
                nc.gpsimd.memset(acc_a[:, :], 0.0)
            if nd == 0:
                nc.gpsimd.memset(acc_d[:, :], 0.0)

            scr_a = sb.tile([128, min(q, CHUNK)], bf16)
            scr_d = sb.tile([128, min(q, CHUNK)], f32)

            k = 0
            for b in range(nblk):
                for c0, fd in chunks:
                    eng, idx = assign[k]
                    k += 1
                    if eng == "A":
                        nc.scalar.activation(
                            scr_a[:, :fd],
                            v_rep[:, c0 : c0 + fd],
                            mybir.ActivationFunctionType.Relu,
                            bias=ub[:, b : b + 1],
                            accum_out=acc_a[:, idx : idx + 1],
                        )
                    else:
                        nc.vector.tensor_scalar(
                            scr_d[:, :fd],
                            v_rep[:, c0 : c0 + fd],
                            negu[:, b : b + 1],
                            None,
                            op0=mybir.AluOpType.max,
                            op1=mybir.AluOpType.add,
                            accum_out=acc_d[:, idx : idx + 1],
                        )

            nc.sync.dma_start(out=out_t[:, :na_c], in_=acc_a[:, :])
            nc.sync.dma_start(out=out_t[:, na_c:], in_=acc_d[:, :])

    nc.finalize()
    _prog_cache[key] = (nc, assign, na_c, nd_c)
    return _prog_cache[key]


def kernel(preds: np.ndarray, targets: np.ndarray) -> np.ndarray:
    global LAST_EXEC_NS, LAST_RESULTS

    p = np.asarray(preds, dtype=np.float32).reshape(-1)
    t = np.asarray(targets).reshape(-1)

    u = (1.0 - p[t == 1]).astype(np.float32)  # positive side
    v = p[t == 0].astype(np.float32)  # negative side
    nu, nv = u.size, v.size

    # Pick the row side (sharded across cores) to minimize per-core pair count.
    def cost(nrows, ncols):
        nblk = max(1, math.ceil(nrows / (128 * N_CORES)))
        q = max(128, 128 * math.ceil(ncols / 128))
        return nblk * 128 * q, nblk, q

    cost_u, nblk_u, q_u = cost(nu, nv)
    cost_v, nblk_v, q_v = cost(nv, nu)
    if cost_u <= cost_v:
        rows, cols, nblk, q = u, v, nblk_u, q_u
        n_rows_real, n_cols_real = nu, nv
    else:
        rows, cols, nblk, q = v, u, nblk_v, q_v
        n_rows_real, n_cols_real = nv, nu

    rtot = nblk * 128 * N_CORES
    nreal = rows.size
    rows_pad = np.zeros(rtot, dtype=np.float32)
    rows_pad[:nreal] = rows
    cols_big = np.full(q, NEG_BIG, dtype=np.float32)
    cols_big[: cols.size] = cols

    # per-core [128, nblk] layouts: element (p, b) = row b*128 + p of the slice
    ub_all = np.full(rtot, NEG_BIG, dtype=np.float32)
    ub_all[:nreal] = rows
    negu_all = np.zeros(rtot, dtype=np.float32)
    negu_all[:nreal] = -rows

    (nc, assign, na_c, nd_c) = _build(nblk, q)

    per = nblk * 128
    in_maps = []
    for c in range(N_CORES):
        sl = slice(c * per, (c + 1) * per)
        in_maps.append(
            {
                "cols_big": cols_big,
                "ub": np.ascontiguousarray(ub_all[sl].reshape(nblk, 128).T),
                "negu": np.ascontiguousarray(negu_all[sl].reshape(nblk, 128).T),
            }
        )

    br = run_bass_kernel_spmd(nc, in_maps, list(range(N_CORES)), trace=TRACE)
    results = br.results
    LAST_EXEC_NS = getattr(br, "exec_time_ns", None)
    LAST_RESULTS = br

    chunks = _chunks(q)
    relu_sum = 0.0
    rows64 = rows_pad.astype(np.float64)
    for c in range(N_CORES):
        o = np.asarray(results[c]["out"], dtype=np.float64)
        acc_a, acc_d = o[:, :na_c], o[:, na_c:]
        base = c * per
        k = 0
        for b in range(nblk):
            # real rows in this block: global rows [base+b*128, base+b*128+128)
            lo = base + b * 128
            n_real_p = min(max(nreal - lo, 0), 128)
            for c0, fd in chunks:
                eng, idx = assign[k]
                k += 1
                if eng == "A":
                    # padded rows/cols contribute exactly 0
                    relu_sum += acc_a[:, idx].sum()
                elif n_real_p > 0:
                    # sum over real rows of (acc + fd*u_p); padded cols inside
                    # acc contribute -u_p each, cancelled exactly by +fd*u_p
                    seg = acc_d[:n_real_p, idx]
                    useg = rows64[lo : lo + n_real_p]
                    relu_sum += seg.sum() + fd * useg.sum()

    u64 = u.astype(np.float64)
    v64 = v.astype(np.float64)
    sq_sum = (
        nv * (u64 * u64).sum()
        + 2.0 * u64.sum() * v64.sum()
        + nu * (v64 * v64).sum()
    )
    num_pairs = np.float64(nu) * np.float64(nv)
    with np.errstate(divide="ignore", invalid="ignore"):
        loss = np.float32((sq_sum + MARGIN * relu_sum) / num_pairs)
    return np.asarray(loss, dtype=np.float32)


# revision 12
# speedup vs baseline: 3.6448x; 1.1893x over previous
"""AUCM loss kernel for Trainium2 (8 NeuronCores, Bass/Tile).

Reference math (N = 16384 preds, int32 targets):
    pos = preds[targets==1]; neg = preds[targets==0]
    d_ij = 1 - (pos_i - neg_j)
    loss = mean_ij [ d_ij^2 + MARGIN*relu(d_ij) ]

Decomposition: with u_i = 1 - pos_i and v_j = neg_j, d_ij = u_i + v_j.
    sum_ij d^2     = Nv*sum(u^2) + 2*sum(u)*sum(v) + Nu*sum(v^2)  (host, O(N))
    sum_ij relu(d) = the real O(Nu*Nv) work -> computed on device.

Device strategy (no TensorEngine at all — ScalarE and VectorE both stream the
pairwise grid directly out of SBUF, which beats producing D tiles on the PE
since the PE's cold-clock column rate equals just one consumer's rate):
  - v is DMA-broadcast to all 128 partitions: v_rep [128, q] fp32.
  - A 128-row block of u lives as one column u_col [128,1].
  - ScalarE, one instruction per block:
        ACTIVATE(Relu, in=v_rep, bias=u_col, accum_out)  ->
        per-partition sum_j relu(v_j + u_p); -1e30 padding (rows or cols)
        makes relu() return 0 for any padded pair.
  - VectorE, one instruction per block, using
        relu(v + u) = u + max(v, -u):
        TENSOR_SCALAR(max, scalar1=-u_col, reduce=add, accum_out) ->
        per-partition sum_j max(v_j, -u_p). Single-source fp32 SBUF runs in
        the DVE 2x port mode (2 elem/lane/cyc). The host adds the q*u_p
        correction in float64 (padded cols contribute max(-1e30,-u) = -u,
        cancelled exactly by +u; padded rows are dropped on the host).
  - Each block's accum lands in its own column of acc_a/acc_d; both matrices
    are DMA'd out raw [128, na+nd] and the host does the final combine.

Sharding: the longer of (pos, neg) becomes the row side, split evenly across
the 8 cores (each core gets nblk 128-row blocks); the col side is replicated.
"""

import math
import os
import sys

import numpy as np

for _p in ("/opt/trn_rl_repo", "/root/.axon_site/_ro/trn_rl_repo"):
    if os.path.isdir(_p) and _p not in sys.path:
        sys.path.append(_p)

import concourse.bacc as bacc
import concourse.bass as bass
import concourse.tile as tile
from concourse import mybir
from concourse.bass_utils import run_bass_kernel_spmd

N_CORES = 8
MARGIN = 1.0
NEG_BIG = -1.0e30
CHUNK = 4096  # max free-dim per consumer instruction

# test-harness hooks (the grading path never touches these)
TRACE = False
LAST_EXEC_NS = None
LAST_RESULTS = None

_prog_cache: dict = {}


def _chunks(q):
    out = []
    c0 = 0
    while c0 < q:
        fd = min(CHUNK, q - c0)
        out.append((c0, fd))
        c0 += fd
    return out


def _assign(nblk, chunks):
    """Greedy ACT/DVE split by modeled per-instruction cost (ns)."""

    def act_cost(fd):
        return (680.0 + fd) / 1.2 + 230.0  # ACTIVATE + READ_ACCUMULATOR (measured)

    def dve_cost(fd):
        return (500.0 + fd) / 0.96  # accum op runs 1x (measured)

    load_a = load_d = 0.0
    assign = []
    na = nd = 0
    for _b in range(nblk):
        for _c0, fd in chunks:
            if load_a + act_cost(fd) <= load_d + dve_cost(fd):
                assign.append(("A", na))
                na += 1
                load_a += act_cost(fd)
            else:
                assign.append(("D", nd))
                nd += 1
                load_d += dve_cost(fd)
    return assign, na, nd


def _build(nblk, q):
    """Bass program for one core: nblk 128-row blocks x q cols."""
    key = (nblk, q)
    if key in _prog_cache:
        return _prog_cache[key]

    f32 = mybir.dt.float32
    bf16 = mybir.dt.bfloat16
    chunks = _chunks(q)
    assign, na, nd = _assign(nblk, chunks)
    na_c, nd_c = max(1, na), max(1, nd)

    nc = bacc.Bacc(None, target_bir_lowering=False)
    cols_big = nc.dram_tensor("cols_big", [q], f32, kind="ExternalInput")
    ub_t = nc.dram_tensor("ub", [128, nblk], f32, kind="ExternalInput")
    negu_t = nc.dram_tensor("negu", [128, nblk], f32, kind="ExternalInput")
    out_t = nc.dram_tensor("out", [128, na_c + nd_c], f32, kind="ExternalOutput")

    with tile.TileContext(nc) as tc:
        with tc.tile_pool(name="sb", bufs=1) as sb:
            v_rep = sb.tile([128, q], f32)
            h = cols_big[:]
            nstripes = max(1, min(8, q // 128))
            sw = (q // nstripes + 127) // 128 * 128
            c0 = 0
            while c0 < q:
                w = min(sw, q - c0)
                bc = bass.AP(
                    tensor=h.tensor, offset=h.offset + c0, ap=[[0, 128], [1, w]]
                )
                nc.sync.dma_start(out=v_rep[:, c0 : c0 + w], in_=bc)
                c0 += w
            ub = sb.tile([128, nblk], f32)
            nc.sync.dma_start(out=ub[:, :], in_=ub_t[:, :])
            negu = sb.tile([128, nblk], f32)
            nc.sync.dma_start(out=negu[:, :], in_=negu_t[:, :])

            acc_a = sb.tile([128, na_c], f32)
            acc_d = sb.tile([128, nd_c], f32)
            if na == 0:
                nc.gpsimd.memset(acc_a[:, :], 0.0)
            if nd == 0:
                nc.gpsimd.memset(acc_d[:, :], 0.0)

            scr_a = sb.tile([128, min(q, CHUNK)], bf16)
            scr_d = sb.tile([128, min(q, CHUNK)], f32)

            k = 0
            for b in range(nblk):
                for c0, fd in chunks:
                    eng, idx = assign[k]
                    k += 1
                    if eng == "A":
                        nc.scalar.activation(
                            scr_a[:, :fd],
                            v_rep[:, c0 : c0 + fd],
                            mybir.ActivationFunctionType.Relu,
                            bias=ub[:, b : b + 1],
                            accum_out=acc_a[:, idx : idx + 1],
                        )
                    else:
                        nc.vector.tensor_scalar(
                            scr_d[:, :fd],
                            v_rep[:, c0 : c0 + fd],
                            negu[:, b : b + 1],
                            None,
                            op0=mybir.AluOpType.max,
                            op1=mybir.AluOpType.add,
                            accum_out=acc_d[:, idx : idx + 1],
                        )

            nc.sync.dma_start(out=out_t[:, :na_c], in_=acc_a[:, :])
            nc.sync.dma_start(out=out_t[:, na_c:], in_=acc_d[:, :])

    nc.finalize()
    _prog_cache[key] = (nc, assign, na_c, nd_c)
    return _prog_cache[key]


def kernel(preds: np.ndarray, targets: np.ndarray) -> np.ndarray:
    global LAST_EXEC_NS, LAST_RESULTS

    p = np.asarray(preds, dtype=np.float32).reshape(-1)
    t = np.asarray(targets).reshape(-1)

    u = (1.0 - p[t == 1]).astype(np.float32)  # positive side
    v = p[t == 0].astype(np.float32)  # negative side
    nu, nv = u.size, v.size

    # Pick the row side (sharded across cores) to minimize per-core pair count.
    def cost(nrows, ncols):
        nblk = max(1, math.ceil(nrows / (128 * N_CORES)))
        q = max(128, 128 * math.ceil(ncols / 128))
        return nblk * 128 * q, nblk, q

    cost_u, nblk_u, q_u = cost(nu, nv)
    cost_v, nblk_v, q_v = cost(nv, nu)
    if cost_u <= cost_v:
        rows, cols, nblk, q = u, v, nblk_u, q_u
        n_rows_real, n_cols_real = nu, nv
    else:
        rows, cols, nblk, q = v, u, nblk_v, q_v
        n_rows_real, n_cols_real = nv, nu

    rtot = nblk * 128 * N_CORES
    nreal = rows.size
    rows_pad = np.zeros(rtot, dtype=np.float32)
    rows_pad[:nreal] = rows
    cols_big = np.full(q, NEG_BIG, dtype=np.float32)
    cols_big[: cols.size] = cols

    # per-core [128, nblk] layouts: element (p, b) = row b*128 + p of the slice
    ub_all = np.full(rtot, NEG_BIG, dtype=np.float32)
    ub_all[:nreal] = rows
    negu_all = np.zeros(rtot, dtype=np.float32)
    negu_all[:nreal] = -rows

    (nc, assign, na_c, nd_c) = _build(nblk, q)

    per = nblk * 128
    in_maps = []
    for c in range(N_CORES):
        sl = slice(c * per, (c + 1) * per)
        in_maps.append(
            {
                "cols_big": cols_big,
                "ub": np.ascontiguousarray(ub_all[sl].reshape(nblk, 128).T),
                "negu": np.ascontiguousarray(negu_all[sl].reshape(nblk, 128).T),
            }
        )

    br = run_bass_kernel_spmd(nc, in_maps, list(range(N_CORES)), trace=TRACE)
    results = br.results
    LAST_EXEC_NS = getattr(br, "exec_time_ns", None)
    LAST_RESULTS = br

    chunks = _chunks(q)
    relu_sum = 0.0
    rows64 = rows_pad.astype(np.float64)
    for c in range(N_CORES):
        o = np.asarray(results[c]["out"], dtype=np.float64)
        acc_a, acc_d = o[:, :na_c], o[:, na_c:]
        base = c * per
        k = 0
        for b in range(nblk):
            # real rows in this block: global rows [base+b*128, base+b*128+128)
            lo = base + b * 128
            n_real_p = min(max(nreal - lo, 0), 128)
            for c0, fd in chunks:
                eng, idx = assign[k]
                k += 1
                if eng == "A":
                    # padded rows/cols contribute exactly 0
                    relu_sum += acc_a[:, idx].sum()
                elif n_real_p > 0:
                    # sum over real rows of (acc + fd*u_p); padded cols inside
                    # acc contribute -u_p each, cancelled exactly by +fd*u_p
                    seg = acc_d[:n_real_p, idx]
                    useg = rows64[lo : lo + n_real_p]
                    relu_sum += seg.sum() + fd * useg.sum()

    u64 = u.astype(np.float64)
    v64 = v.astype(np.float64)
    sq_sum = (
        nv * (u64 * u64).sum()
        + 2.0 * u64.sum() * v64.sum()
        + nu * (v64 * v64).sum()
    )
    num_pairs = np.float64(nu) * np.float64(nv)
    with np.errstate(divide="ignore", invalid="ignore"):
        loss = np.float32((sq_sum + MARGIN * relu_sum) / num_pairs)
    return np.asarray(loss, dtype=np.float32)


# revision 19
# speedup vs baseline: 3.8520x; 1.0568x over previous
"""AUCM loss kernel for Trainium2 (8 NeuronCores, raw Bass).

Reference math (N = 16384 preds, int32 targets):
    pos = preds[targets==1]; neg = preds[targets==0]
    d_ij = 1 - (pos_i - neg_j)
    loss = mean_ij [ d_ij^2 + MARGIN*relu(d_ij) ]

Decomposition: with u_i = 1 - pos_i and v_j = neg_j, d_ij = u_i + v_j.
    sum_ij d^2     = Nv*sum(u^2) + 2*sum(u)*sum(v) + Nu*sum(v^2)  (host, O(N))
    sum_ij relu(d) = the real O(Nu*Nv) work -> computed on device.

Device strategy (no TensorEngine; ScalarE and VectorE both stream the
pairwise grid directly out of SBUF — an explicit PE-built D matrix would cap
both consumers at the PE's own column rate):
  - v is DMA-broadcast to all 128 partitions: v_rep [128, q] fp32 (striped
    over 8 DMA queues).
  - A 128-row block of u lives as one column u_col [128,1].
  - ScalarE, one instruction per (block, chunk):
        ACTIVATE(Relu, in=v_rep, bias=u_col, accum_out) ->
        per-partition sum_j relu(v_j + u_p); -1e30 padding (rows or cols)
        makes relu() return 0 for any padded pair.
  - VectorE, one instruction per (block, chunk), using
        relu(v + u) = u + max(v, -u):
        TENSOR_SCALAR(max, scalar1=-u_col, reduce=add, accum_out) ->
        per-partition sum_j max(v_j, -u_p). The host adds the fd*u_p
        correction in float64 (padded cols contribute max(-1e30,-u) = -u,
        cancelled exactly by +u; padded rows are dropped on the host).
  - Each unit's accum lands in its own column of acc_a/acc_d; both matrices
    are DMA'd out raw and the host does the final combine.

The kernel is raw Bass (no TileContext): a 3-engine pipeline with two
semaphores (dma_in, acc_done). This avoids Tile's multi-microsecond
semaphore-init preamble and end-of-kernel barrier butterfly.

Sharding: the longer of (pos, neg) becomes the row side, split evenly across
the 8 cores (each core gets nblk 128-row blocks); the col side is replicated.
"""

import math
import os
import sys

import numpy as np

for _p in ("/opt/trn_rl_repo", "/root/.axon_site/_ro/trn_rl_repo"):
    if os.path.isdir(_p) and _p not in sys.path:
        sys.path.append(_p)

import concourse.bacc as bacc
import concourse.bass as bass
from concourse import mybir
from concourse.bass_utils import run_bass_kernel_spmd

N_CORES = 8
MARGIN = 1.0
NEG_BIG = -1.0e30
CHUNK = 4096  # max free-dim per consumer instruction

# test-harness hooks (the grading path never touches these)
TRACE = False
LAST_EXEC_NS = None
LAST_RESULTS = None

_prog_cache: dict = {}


def _chunks(q):
    out = []
    c0 = 0
    while c0 < q:
        fd = min(CHUNK, q - c0)
        out.append((c0, fd))
        c0 += fd
    return out


def _units(nblk, q):
    """(block, c0, fd) units; guarantees at least one unit per engine."""
    chunks = _chunks(q)
    units = [(b, c0, fd) for b in range(nblk) for c0, fd in chunks]
    if len(units) == 1:
        b, c0, fd = units[0]
        h = max(2, fd // 2) // 2 * 2  # even split
        units = [(b, c0, h), (b, c0 + h, fd - h)]
    return units


def _assign(units):
    """Greedy ACT/DVE split by measured per-instruction cost (ns)."""

    def act_cost(fd):
        return (680.0 + fd) / 1.2 + 230.0  # ACTIVATE + READ_ACCUMULATOR

    def dve_cost(fd):
        return (500.0 + fd) / 0.96

    load_a = load_d = 0.0
    assign = []
    na = nd = 0
    for _b, _c0, fd in units:
        if load_a + act_cost(fd) <= load_d + dve_cost(fd):
            assign.append(("A", na))
            na += 1
            load_a += act_cost(fd)
        else:
            assign.append(("D", nd))
            nd += 1
            load_d += dve_cost(fd)
    return assign, na, nd


def _build(nblk, q):
    """Raw Bass program for one core: nblk 128-row blocks x q cols."""
    key = (nblk, q)
    if key in _prog_cache:
        return _prog_cache[key]

    f32 = mybir.dt.float32
    bf16 = mybir.dt.bfloat16
    units = _units(nblk, q)
    assign, na, nd = _assign(units)
    assert na >= 1 and nd >= 1

    nc = bacc.Bacc(None, target_bir_lowering=False)
    cols_big = nc.dram_tensor("cols_big", [q], f32, kind="ExternalInput")
    ub_t = nc.dram_tensor("ub", [128, nblk], f32, kind="ExternalInput")
    negu_t = nc.dram_tensor("negu", [128, nblk], f32, kind="ExternalInput")
    out_t = nc.dram_tensor("out", [128, na + nd], f32, kind="ExternalOutput")

    # input DMA plan: v_rep stripes + ub + negu
    nstripes = max(1, min(8, q // 128))
    sw = (q // nstripes + 127) // 128 * 128
    stripes = []
    c0 = 0
    while c0 < q:
        stripes.append((c0, min(sw, q - c0)))
        c0 += sw
    n_in = len(stripes) + 2

    # Each unit gets a private scratch slice (the engines' main outputs are
    # dead stores — only accum_out matters — but same-engine WAW reuse is
    # unsafe on deep pipelines and trips the race detector).
    offs = []
    scr_w = 1  # slot 0 reserved for the table-load dummy
    for _b, _c0, fd in units:
        offs.append(scr_w)
        scr_w += fd

    with (
        nc.sbuf_tensor([128, q], f32) as v_rep,
        nc.sbuf_tensor([128, nblk], f32) as ub_sb,
        nc.sbuf_tensor([128, nblk], f32) as negu_sb,
        nc.sbuf_tensor([128, na], f32) as acc_a,
        nc.sbuf_tensor([128, nd], f32) as acc_d,
        nc.sbuf_tensor([128, scr_w], bf16) as scr,
        nc.semaphore("dma_in") as dma_in,
        nc.semaphore("acc_done") as acc_done,
        nc.Block() as block,
    ):

        @block.sync
        def _(sync: bass.BassEngine):
            h = cols_big[:]
            for c0, w in stripes:
                bc = bass.AP(
                    tensor=h.tensor, offset=h.offset + c0, ap=[[0, 128], [1, w]]
                )
                sync.dma_start(out=v_rep[:, c0 : c0 + w], in_=bc).then_inc(dma_in, 16)
            with nc.allow_non_contiguous_dma(reason="tiny [128, nblk] u tiles"):
                sync.dma_start(out=ub_sb[:, :], in_=ub_t[:, :]).then_inc(dma_in, 16)
                sync.dma_start(out=negu_sb[:, :], in_=negu_t[:, :]).then_inc(
                    dma_in, 16
                )
            # wait for both consumers, then write results out
            sync.wait_ge(acc_done, 2)
            with nc.allow_non_contiguous_dma(reason="small accum outputs"):
                sync.dma_start(out=out_t[:, :na], in_=acc_a[:, :]).then_inc(dma_in, 16)
                sync.dma_start(out=out_t[:, na:], in_=acc_d[:, :]).then_inc(dma_in, 16)
            sync.wait_ge(dma_in, 16 * (n_in + 2))

        @block.scalar
        def _(scalar: bass.BassEngine):
            # dummy activation: hoists the ~1.5us ACT_TABLE_LOAD before the
            # DMA wait so it overlaps the input transfer
            zero = nc.const_aps.scalar_like(0.0, scr[:, 0:1])
            scalar.activation(scr[:, 0:1], zero, mybir.ActivationFunctionType.Relu)
            scalar.wait_ge(dma_in, 16 * n_in)
            seen = 0
            for k, (b, c0, fd) in enumerate(units):
                eng, idx = assign[k]
                if eng != "A":
                    continue
                seen += 1
                ins = scalar.activation(
                    scr[:, offs[k] : offs[k] + fd],
                    v_rep[:, c0 : c0 + fd],
                    mybir.ActivationFunctionType.Relu,
                    bias=ub_sb[:, b : b + 1],
                    accum_out=acc_a[:, idx : idx + 1],
                )
                if seen == na:
                    ins.then_inc(acc_done, 1)

        @block.vector
        def _(vector: bass.BassEngine):
            vector.wait_ge(dma_in, 16 * n_in)
            seen = 0
            for k, (b, c0, fd) in enumerate(units):
                eng, idx = assign[k]
                if eng != "D":
                    continue
                seen += 1
                ins = vector.tensor_scalar(
                    scr[:, offs[k] : offs[k] + fd],
                    v_rep[:, c0 : c0 + fd],
                    negu_sb[:, b : b + 1],
                    None,
                    op0=mybir.AluOpType.max,
                    op1=mybir.AluOpType.add,
                    accum_out=acc_d[:, idx : idx + 1],
                )
                if seen == nd:
                    ins.then_inc(acc_done, 1)

    nc.finalize()
    _prog_cache[key] = (nc, units, assign, na, nd)
    return _prog_cache[key]


def kernel(preds: np.ndarray, targets: np.ndarray) -> np.ndarray:
    global LAST_EXEC_NS, LAST_RESULTS

    p = np.asarray(preds, dtype=np.float32).reshape(-1)
    t = np.asarray(targets).reshape(-1)

    u = (1.0 - p[t == 1]).astype(np.float32)  # positive side
    v = p[t == 0].astype(np.float32)  # negative side
    nu, nv = u.size, v.size

    # Pick the row side (sharded across cores) to minimize per-core pair count.
    def cost(nrows, ncols):
        nblk = max(1, math.ceil(nrows / (128 * N_CORES)))
        q = max(128, 128 * math.ceil(ncols / 128))
        return nblk * 128 * q, nblk, q

    cost_u, nblk_u, q_u = cost(nu, nv)
    cost_v, nblk_v, q_v = cost(nv, nu)
    if cost_u <= cost_v:
        rows, cols, nblk, q = u, v, nblk_u, q_u
        n_rows_real, n_cols_real = nu, nv
    else:
        rows, cols, nblk, q = v, u, nblk_v, q_v
        n_rows_real, n_cols_real = nv, nu

    rtot = nblk * 128 * N_CORES
    nreal = rows.size
    rows_pad = np.zeros(rtot, dtype=np.float32)
    rows_pad[:nreal] = rows
    cols_big = np.full(q, NEG_BIG, dtype=np.float32)
    cols_big[: cols.size] = cols

    # per-core [128, nblk] layouts: element (p, b) = row b*128 + p of the slice
    ub_all = np.full(rtot, NEG_BIG, dtype=np.float32)
    ub_all[:nreal] = rows
    negu_all = np.zeros(rtot, dtype=np.float32)
    negu_all[:nreal] = -rows

    (nc, units, assign, na, nd) = _build(nblk, q)

    per = nblk * 128
    in_maps = []
    for c in range(N_CORES):
        sl = slice(c * per, (c + 1) * per)
        in_maps.append(
            {
                "cols_big": cols_big,
                "ub": np.ascontiguousarray(ub_all[sl].reshape(nblk, 128).T),
                "negu": np.ascontiguousarray(negu_all[sl].reshape(nblk, 128).T),
            }
        )

    br = run_bass_kernel_spmd(nc, in_maps, list(range(N_CORES)), trace=TRACE)
    results = br.results
    LAST_EXEC_NS = getattr(br, "exec_time_ns", None)
    LAST_RESULTS = br

    relu_sum = 0.0
    rows64 = rows_pad.astype(np.float64)
    for c in range(N_CORES):
        o = np.asarray(results[c]["out"], dtype=np.float64)
        acc_a, acc_d = o[:, :na], o[:, na:]
        base = c * per
        for k, (b, c0, fd) in enumerate(units):
            eng, idx = assign[k]
            lo = base + b * 128
            n_real_p = min(max(nreal - lo, 0), 128)
            if eng == "A":
                # padded rows/cols contribute exactly 0
                relu_sum += acc_a[:, idx].sum()
            elif n_real_p > 0:
                # sum over real rows of (acc + fd*u_p); padded cols inside
                # acc contribute -u_p each, cancelled exactly by +fd*u_p
                seg = acc_d[:n_real_p, idx]
                useg = rows64[lo : lo + n_real_p]
                relu_sum += seg.sum() + fd * useg.sum()

    u64 = u.astype(np.float64)
    v64 = v.astype(np.float64)
    sq_sum = (
        nv * (u64 * u64).sum() + 2.0 * u64.sum() * v64.sum() + nu * (v64 * v64).sum()
    )
    num_pairs = np.float64(nu) * np.float64(nv)
    with np.errstate(divide="ignore", invalid="ignore"):
        loss = np.float32((sq_sum + MARGIN * relu_sum) / num_pairs)
    return np.asarray(loss, dtype=np.float32)


# revision 23
# speedup vs baseline: 4.0966x; 1.0635x over previous
"""AUCM loss kernel for Trainium2 (8 NeuronCores, raw Bass).

Reference math (N = 16384 preds, int32 targets):
    pos = preds[targets==1]; neg = preds[targets==0]
    d_ij = 1 - (pos_i - neg_j)
    loss = mean_ij [ d_ij^2 + MARGIN*relu(d_ij) ]

Decomposition: with u_i = 1 - pos_i and v_j = neg_j, d_ij = u_i + v_j.
    sum_ij d^2     = Nv*sum(u^2) + 2*sum(u)*sum(v) + Nu*sum(v^2)  (host, O(N))
    sum_ij relu(d) = the real O(Nu*Nv) work -> computed on device.

Device strategy (no TensorEngine; ScalarE and VectorE both stream the
pairwise grid directly out of SBUF — an explicit PE-built D matrix would cap
both consumers at the PE's own column rate):
  - v is DMA-broadcast to all 128 partitions: v_rep [128, q] fp32 (striped
    over 8 DMA queues).
  - A 128-row block of u lives as one column u_col [128,1].
  - ScalarE, one instruction per (block, chunk):
        ACTIVATE(Relu, in=v_rep, bias=u_col, accum_out) ->
        per-partition sum_j relu(v_j + u_p); -1e30 padding (rows or cols)
        makes relu() return 0 for any padded pair.
  - VectorE, one instruction per (block, chunk), using
        relu(v + u) = u + max(v, -u):
        TENSOR_SCALAR(max, scalar1=-u_col, reduce=add, accum_out) ->
        per-partition sum_j max(v_j, -u_p). The host adds the fd*u_p
        correction in float64 (padded cols contribute max(-1e30,-u) = -u,
        cancelled exactly by +u; padded rows are dropped on the host).
  - Each unit's accum lands in its own column of acc_a/acc_d; both matrices
    are DMA'd out raw and the host does the final combine.

The kernel is raw Bass (no TileContext): a 3-engine pipeline with two
semaphores (dma_in, acc_done). This avoids Tile's multi-microsecond
semaphore-init preamble and end-of-kernel barrier butterfly.

Sharding: the longer of (pos, neg) becomes the row side, split evenly across
the 8 cores (each core gets nblk 128-row blocks); the col side is replicated.
"""

import math
import os
import sys

import numpy as np

for _p in ("/opt/trn_rl_repo", "/root/.axon_site/_ro/trn_rl_repo"):
    if os.path.isdir(_p) and _p not in sys.path:
        sys.path.append(_p)

import concourse.bacc as bacc
import concourse.bass as bass
from concourse import mybir
from concourse.bass_utils import run_bass_kernel_spmd

N_CORES = 8
MARGIN = 1.0
NEG_BIG = -1.0e30
CHUNK = 4096  # max free-dim per consumer instruction

# test-harness hooks (the grading path never touches these)
TRACE = False
LAST_EXEC_NS = None
LAST_RESULTS = None

_prog_cache: dict = {}


def _chunks(q):
    out = []
    c0 = 0
    while c0 < q:
        fd = min(CHUNK, q - c0)
        out.append((c0, fd))
        c0 += fd
    return out


def _units(nblk, q):
    """(block, c0, fd) units; guarantees at least one unit per engine."""
    chunks = _chunks(q)
    units = [(b, c0, fd) for b in range(nblk) for c0, fd in chunks]
    if len(units) == 1:
        b, c0, fd = units[0]
        h = max(2, fd // 2) // 2 * 2  # even split
        units = [(b, c0, h), (b, c0 + h, fd - h)]
    return units


def _assign(units):
    """Greedy ACT/DVE split by measured per-instruction cost (ns)."""

    def act_cost(fd):
        return (680.0 + fd) / 1.2 + 230.0  # ACTIVATE + READ_ACCUMULATOR

    def dve_cost(fd):
        return (500.0 + fd) / 0.96

    load_a = load_d = 0.0
    assign = []
    na = nd = 0
    for _b, _c0, fd in units:
        if load_a + act_cost(fd) <= load_d + dve_cost(fd):
            assign.append(("A", na))
            na += 1
            load_a += act_cost(fd)
        else:
            assign.append(("D", nd))
            nd += 1
            load_d += dve_cost(fd)
    return assign, na, nd


def _build(nblk, q):
    """Raw Bass program for one core: nblk 128-row blocks x q cols."""
    key = (nblk, q)
    if key in _prog_cache:
        return _prog_cache[key]

    f32 = mybir.dt.float32
    bf16 = mybir.dt.bfloat16
    units = _units(nblk, q)
    assign, na, nd = _assign(units)
    assert na >= 1 and nd >= 1

    nc = bacc.Bacc(None, target_bir_lowering=False)
    cols_big = nc.dram_tensor("cols_big", [q], f32, kind="ExternalInput")
    uu_t = nc.dram_tensor("uu", [128, 2 * nblk], f32, kind="ExternalInput")
    out_t = nc.dram_tensor("out", [128, na + nd], f32, kind="ExternalOutput")

    # input DMA plan: v_rep stripes (issued from 4 different sequencers so the
    # ~600ns-per-DMA descriptor writes don't serialize) + one u-tile DMA
    nstripes = max(1, min(4, q // 128))
    sw = (q // nstripes + 127) // 128 * 128
    stripes = []
    c0 = 0
    while c0 < q:
        stripes.append((c0, min(sw, q - c0)))
        c0 += sw
    n_in = len(stripes) + 1

    # Each unit gets a private scratch slice (the engines' main outputs are
    # dead stores — only accum_out matters — but same-engine WAW reuse is
    # unsafe on deep pipelines and trips the race detector).
    offs = []
    scr_w = 1  # slot 0 reserved for the table-load dummy
    for _b, _c0, fd in units:
        offs.append(scr_w)
        scr_w += fd

    with (
        nc.sbuf_tensor([128, q], f32) as v_rep,
        nc.sbuf_tensor([128, 2 * nblk], f32) as uu_sb,
        nc.sbuf_tensor([128, na + nd], f32) as acc,
        nc.sbuf_tensor([128, scr_w], bf16) as scr,
        nc.semaphore("dma_in") as dma_in,
        nc.semaphore("acc_done") as acc_done,
        nc.Block() as block,
    ):
        ub_sb = uu_sb[:, :nblk]
        negu_sb = uu_sb[:, nblk:]
        h = cols_big[:]

        def stripe_dma(eng, s):
            c0, w = stripes[s]
            bc = bass.AP(tensor=h.tensor, offset=h.offset + c0, ap=[[0, 128], [1, w]])
            eng.dma_start(out=v_rep[:, c0 : c0 + w], in_=bc).then_inc(dma_in, 16)

        # stripe issuers: only sync/scalar (HWDGE) and gpsimd (SWDGE) can DMA
        issuers = {}
        for s in range(len(stripes)):
            issuers.setdefault(["scalar", "sync", "gpsimd", "scalar"][s % 4], []).append(s)

        @block.gpsimd
        def _(gpsimd: bass.BassEngine):
            for s in issuers.get("gpsimd", []):
                stripe_dma(gpsimd, s)

        @block.sync
        def _(sync: bass.BassEngine):
            for s in issuers.get("sync", []):
                stripe_dma(sync, s)
            with nc.allow_non_contiguous_dma(reason="tiny [128, 2*nblk] u tile"):
                sync.dma_start(out=uu_sb[:, :], in_=uu_t[:, :]).then_inc(dma_in, 16)
            # wait for both consumers, then write results out
            sync.wait_ge(acc_done, 2)
            with nc.allow_non_contiguous_dma(reason="small accum outputs"):
                sync.dma_start(out=out_t[:, :], in_=acc[:, :]).then_inc(dma_in, 16)
            sync.wait_ge(dma_in, 16 * (n_in + 1))

        @block.scalar
        def _(scalar: bass.BassEngine):
            # dummy activation: hoists the ~1.5us ACT_TABLE_LOAD before the
            # DMA wait so it overlaps the input transfer
            zero = nc.const_aps.scalar_like(0.0, scr[:, 0:1])
            scalar.activation(scr[:, 0:1], zero, mybir.ActivationFunctionType.Relu)
            for s in issuers.get("scalar", []):
                stripe_dma(scalar, s)
            scalar.wait_ge(dma_in, 16 * n_in)
            seen = 0
            for k, (b, c0, fd) in enumerate(units):
                eng, idx = assign[k]
                if eng != "A":
                    continue
                seen += 1
                ins = scalar.activation(
                    scr[:, offs[k] : offs[k] + fd],
                    v_rep[:, c0 : c0 + fd],
                    mybir.ActivationFunctionType.Relu,
                    bias=ub_sb[:, b : b + 1],
                    accum_out=acc[:, idx : idx + 1],
                )
                if seen == na:
                    ins.then_inc(acc_done, 1)

        @block.vector
        def _(vector: bass.BassEngine):
            for s in issuers.get("vector", []):
                stripe_dma(vector, s)
            vector.wait_ge(dma_in, 16 * n_in)
            seen = 0
            for k, (b, c0, fd) in enumerate(units):
                eng, idx = assign[k]
                if eng != "D":
                    continue
                seen += 1
                ins = vector.tensor_scalar(
                    scr[:, offs[k] : offs[k] + fd],
                    v_rep[:, c0 : c0 + fd],
                    negu_sb[:, b : b + 1],
                    None,
                    op0=mybir.AluOpType.max,
                    op1=mybir.AluOpType.add,
                    accum_out=acc[:, na + idx : na + idx + 1],
                )
                if seen == nd:
                    ins.then_inc(acc_done, 1)

    nc.finalize()
    _prog_cache[key] = (nc, units, assign, na, nd)
    return _prog_cache[key]


def kernel(preds: np.ndarray, targets: np.ndarray) -> np.ndarray:
    global LAST_EXEC_NS, LAST_RESULTS

    p = np.asarray(preds, dtype=np.float32).reshape(-1)
    t = np.asarray(targets).reshape(-1)

    u = (1.0 - p[t == 1]).astype(np.float32)  # positive side
    v = p[t == 0].astype(np.float32)  # negative side
    nu, nv = u.size, v.size

    # Pick the row side (sharded across cores) to minimize per-core pair count.
    def cost(nrows, ncols):
        nblk = max(1, math.ceil(nrows / (128 * N_CORES)))
        q = max(128, 128 * math.ceil(ncols / 128))
        return nblk * 128 * q, nblk, q

    cost_u, nblk_u, q_u = cost(nu, nv)
    cost_v, nblk_v, q_v = cost(nv, nu)
    if cost_u <= cost_v:
        rows, cols, nblk, q = u, v, nblk_u, q_u
        n_rows_real, n_cols_real = nu, nv
    else:
        rows, cols, nblk, q = v, u, nblk_v, q_v
        n_rows_real, n_cols_real = nv, nu

    rtot = nblk * 128 * N_CORES
    nreal = rows.size
    rows_pad = np.zeros(rtot, dtype=np.float32)
    rows_pad[:nreal] = rows
    cols_big = np.full(q, NEG_BIG, dtype=np.float32)
    cols_big[: cols.size] = cols

    # per-core [128, nblk] layouts: element (p, b) = row b*128 + p of the slice
    ub_all = np.full(rtot, NEG_BIG, dtype=np.float32)
    ub_all[:nreal] = rows
    negu_all = np.zeros(rtot, dtype=np.float32)
    negu_all[:nreal] = -rows

    (nc, units, assign, na, nd) = _build(nblk, q)

    per = nblk * 128
    in_maps = []
    for c in range(N_CORES):
        sl = slice(c * per, (c + 1) * per)
        uu = np.concatenate(
            [ub_all[sl].reshape(nblk, 128).T, negu_all[sl].reshape(nblk, 128).T],
            axis=1,
        )
        in_maps.append(
            {
                "cols_big": cols_big,
                "uu": np.ascontiguousarray(uu),
            }
        )

    br = run_bass_kernel_spmd(nc, in_maps, list(range(N_CORES)), trace=TRACE)
    results = br.results
    LAST_EXEC_NS = getattr(br, "exec_time_ns", None)
    LAST_RESULTS = br

    relu_sum = 0.0
    rows64 = rows_pad.astype(np.float64)
    for c in range(N_CORES):
        o = np.asarray(results[c]["out"], dtype=np.float64)
        acc_a, acc_d = o[:, :na], o[:, na:]
        base = c * per
        for k, (b, c0, fd) in enumerate(units):
            eng, idx = assign[k]
            lo = base + b * 128
            n_real_p = min(max(nreal - lo, 0), 128)
            if eng == "A":
                # padded rows/cols contribute exactly 0
                relu_sum += acc_a[:, idx].sum()
            elif n_real_p > 0:
                # sum over real rows of (acc + fd*u_p); padded cols inside
                # acc contribute -u_p each, cancelled exactly by +fd*u_p
                seg = acc_d[:n_real_p, idx]
                useg = rows64[lo : lo + n_real_p]
                relu_sum += seg.sum() + fd * useg.sum()

    u64 = u.astype(np.float64)
    v64 = v.astype(np.float64)
    sq_sum = (
        nv * (u64 * u64).sum() + 2.0 * u64.sum() * v64.sum() + nu * (v64 * v64).sum()
    )
    num_pairs = np.float64(nu) * np.float64(nv)
    with np.errstate(divide="ignore", invalid="ignore"):
        loss = np.float32((sq_sum + MARGIN * relu_sum) / num_pairs)
    return np.asarray(loss, dtype=np.float32)
